# revision 32
# baseline (speedup 1.0000x reference)
"""Causal MHA with RoPE on 8 trn2 NeuronCores.

Problem: x[2,2048,1024], 16 heads x 64, fp32, causal, RoPE, Wq/Wk/Wv/Wo.

Sharding: core c handles batch b = c//4 and head group g = c%4 (4 heads,
256 feature rows). Each core computes its partial output contribution
out_partial = attnout_g @ Wo[:, g_slice].T of shape [2048, 1024]; the host
sums the 4 partials per batch.

Per-core kernel layout (all "T" tensors are feature-major so the PE
contraction runs over partitions):
  xT   [1024, 2048] = x[b].T
  wqT/wkT/wvT [1024, 256] = W[g_slice, :].T
  woT  [256, 1024]  = Wo[:, g_slice].T
  QT/KT [128, 512]x(2x4)  (two heads per 128-partition tile, RoPE applied)
  V    [128, 65]x16 k-chunks per head-slot ([seq-chunk, head_dim+ones-col];
       the ones column makes the PV matmul also produce softmax denominators)
  scores are computed transposed (S^T[k, q] = K @ Q^T) so softmax sums run
  over the PSUM partition axis via the V ones-column, the causal mask is a
  [128,128] 0/1 multiply on the diagonal blocks only, and strictly-upper
  blocks are skipped entirely.
"""

import numpy as np

B, S, D, H, HD = 2, 2048, 1024, 16, 64
NCORES = 8
GH = 4  # heads per core
GD = GH * HD  # 256
P = 128
NDC = D // P  # 8 feature chunks
NST = S // P  # 16 seq chunks of 128
QW = 512  # q tile width
NQT = S // QW  # 4
NKC = S // P  # 16 k chunks
SCALE = float(HD) ** -0.5

# matmul operand dtype: "f32r" = 4x-rate fp32 mode, "f32" = exact fp32
MM_MODE = "f32r"

_cache = {}


def _install_shims():
    """Make TileContext kernels compile+profile in this environment."""
    import sys
    import types

    if "antenv.axon_hooks" not in sys.modules:
        mod = types.ModuleType("antenv.axon_hooks")
        mod._hook = None

        def set_axon_ntff_profile_hook(h):
            mod._hook = h

        def get_axon_ntff_profile_hook():
            return mod._hook

        mod.set_axon_ntff_profile_hook = set_axon_ntff_profile_hook
        mod.get_axon_ntff_profile_hook = get_axon_ntff_profile_hook
        sys.modules["antenv.axon_hooks"] = mod
        import antenv

        antenv.axon_hooks = mod
        try:
            from trn_agent_boot.trn_boot import _ntff_profile_via_ctypes

            hook = _ntff_profile_via_ctypes("/opt/axon/libaxon_pjrt.so")
            if hook is not None:
                mod.set_axon_ntff_profile_hook(hook)
        except Exception:
            pass
        try:
            import concourse.bass_utils as bu

            bu.upload_artifacts = lambda tmpdir: f"file://{tmpdir}"
        except Exception:
            pass

    import concourse.tile as tile_mod
    import concourse.mybir as mybir
    from concourse.vector_clock import ScopedClock

    if getattr(tile_mod.TileContext, "_tail_drain_patched", False):
        return

    def _drain_and_barrier(self, tick_clock, wait_clock):
        # The image's walrus rejects >1 sync wait per SP CTRL instruction;
        # spread the kernel-tail waits over single-wait NOPs.
        nc = self.nc
        nop = nc.sync.nop(nofuse=True)
        wait_clock.add_sem_waits(nop.ins, ScopedClock({None: tick_clock.global_clock}))
        si = nop.ins.sync_info
        if si is not None and si.on_wait and len(si.on_wait) > 1:
            extra = list(si.on_wait[1:])
            del si.on_wait[1:]
            for w in extra:
                n2 = nc.sync.nop(nofuse=True)
                si2 = n2.ins.sync_info
                if si2 is None:
                    n2.ins.sync_info = mybir.SyncInfo(on_wait=[w], on_update=[])
                else:
                    si2.on_wait.append(w)
        nc.sync.drain()
        nc.all_engine_barrier()
        assert self.sems is not None
        popped = nc._tile_sem_poison_stack.pop()
        assert popped is self._sem_poison
        nc.clear_and_free_semaphores(list(self.sems.allocated().values()))
        nc.all_engine_barrier()

    tile_mod.TileContext._drain_and_barrier = _drain_and_barrier
    tile_mod.TileContext._tail_drain_patched = True


MAX_WAITS = 1  # walrus in this image allows only 1 sync wait per instruction


def _split_excess_waits(nc, max_waits=MAX_WAITS):
    """Spill excess per-instruction sem waits onto same-engine NOPs."""
    import concourse.mybir as mybir

    n = 0
    for f in nc.m.functions:
        for bb in f.blocks:
            new_insts = []
            for inst in bb.instructions:
                si = inst.sync_info
                if si is not None and si.on_wait and len(si.on_wait) > max_waits:
                    extra = list(si.on_wait[: -max_waits])
                    keep = list(si.on_wait[-max_waits:])
                    for i in range(0, len(extra), max_waits):
                        chunk = extra[i : i + max_waits]
                        n += 1
                        nop = mybir.InstNoOp(
                            name=f"waitsplit-{n}",
                            ins=[],
                            outs=[],
                            engine=inst.engine,
                            sync_info=mybir.SyncInfo(on_wait=chunk, on_update=[]),
                        )
                        new_insts.append(nop)
                    del si.on_wait[:]
                    si.on_wait.extend(keep)
                new_insts.append(inst)
            bb.instructions[:] = new_insts
    return n


def build_nc(split_waits=True):
    """Build the per-core Bass program (SPMD: same program on all 8 cores)."""
    key = ("nc", split_waits)
    if key in _cache:
        return _cache[key]
    _install_shims()

    import concourse.bass as bass
    import concourse.mybir as mybir
    import concourse.tile as tile

    dt = mybir.dt
    f32 = dt.float32
    mdt = {"f32r": dt.float32r, "f32": dt.float32}[MM_MODE]

    Exp = mybir.ActivationFunctionType.Exp
    Ln = mybir.ActivationFunctionType.Ln
    AF_Copy = mybir.ActivationFunctionType.Copy
    Alu = mybir.AluOpType

    from concourse import library_config

    nc = bass.Bass()
    xT = nc.dram_tensor("xT", [D, S], mdt, kind="ExternalInput")
    wqT = nc.dram_tensor("wqT", [D, GD], mdt, kind="ExternalInput")
    wkT = nc.dram_tensor("wkT", [D, GD], mdt, kind="ExternalInput")
    wvT = nc.dram_tensor("wvT", [D, GD], mdt, kind="ExternalInput")
    woT = nc.dram_tensor("woT", [GD, D], mdt, kind="ExternalInput")
    cos2 = nc.dram_tensor("cos2", [P, S], f32, kind="ExternalInput")
    sine = nc.dram_tensor("sine", [P, S], f32, kind="ExternalInput")
    dmask = nc.dram_tensor("dmask", [P, P], mdt, kind="ExternalInput")
    onesd = nc.dram_tensor("onesd", [1, HD], mdt, kind="ExternalInput")
    vones = nc.dram_tensor("vones", [P, GH, 1], mdt, kind="ExternalInput")
    zpad = nc.dram_tensor("zpad", [HD, QW], mdt, kind="ExternalInput")
    rmatT = nc.dram_tensor("rmatT", [P, P], mdt, kind="ExternalInput")
    out = nc.dram_tensor("out", [S, D], f32, kind="ExternalOutput")

    with tile.TileContext(nc) as tc:
        with (
            tc.tile_pool(name="const", bufs=1) as cpool,
            tc.tile_pool(name="qkv", bufs=1) as qkvp,
        ):
            # ---- persistent constants (DMAs deferred until after x/w) ----
            cos_sb = cpool.tile([P, S], f32, tag="cos")
            sin_sb = cpool.tile([P, S], f32, tag="sin")
            dm_sb = cpool.tile([P, P], mdt, tag="dm")
            ones1 = cpool.tile([1, HD], mdt, tag="ones")
            rm_sb = cpool.tile([P, P], mdt, tag="rm")
            wo_sb = [
                cpool.tile([P, D], mdt, tag=f"wo{c}", name=f"wo{c}")
                for c in range(2)
            ]

            # ---- Q/K/V storage (persist through attention) ----
            QT = [
                [
                    qkvp.tile([P, QW], mdt, tag=f"qt{th}_{qt}", name=f"qt{th}_{qt}")
                    for qt in range(NQT)
                ]
                for th in range(2)
            ]
            # K^T per head in full-128-partition tiles: head data at its
            # natural row offset, zeros elsewhere, so the scores matmul
            # contracts over 128 partitions (keeps the PE activity monitor
            # at full rate; 64-row contractions never unthrottle the clock)
            KT = [
                [
                    qkvp.tile([P, QW], mdt, tag=f"kt{h}_{qt}", name=f"kt{h}_{qt}")
                    for qt in range(NQT)
                ]
                for h in range(GH)
            ]
            # V chunks: [128 seq, 4 heads x (64 + ones col)]
            VA = [
                qkvp.tile([P, GH * (HD + 1)], mdt, tag=f"va{kc}", name=f"va{kc}")
                for kc in range(NKC)
            ]

            with (
                tc.tile_pool(name="xw", bufs=1) as xwp,
                tc.tile_pool(name="ppsum", bufs=1, space="PSUM") as ppsum,
            ):
                # ---- loads ordered by first use: x0+wq0 (Q d=0), cos/sin
                # (RoPE ~25us), rest of x/wq, wk+zpad (K-proj), wv+V-ones,
                # dmask/ones1/wo (attention / O-proj) ----
                x_sb, w_sb = [], {}
                for d_ in range(NDC):
                    x_sb.append(xwp.tile([P, S], mdt, tag=f"x{d_}", name=f"x{d_}"))
                for wname in ("q", "k", "v"):
                    for d_ in range(NDC):
                        w_sb[wname, d_] = xwp.tile(
                            [P, GD], mdt, tag=f"w{wname}{d_}", name=f"w{wname}{d_}"
                        )
                wmap = {"q": wqT, "k": wkT, "v": wvT}
                nc.sync.dma_start(x_sb[0][:], xT[0:P, :])
                nc.sync.dma_start(w_sb["q", 0][:], wqT[0:P, :])
                nc.sync.dma_start(cos_sb[:], cos2[:])
                nc.sync.dma_start(sin_sb[:], sine[:])
                nc.sync.dma_start(rm_sb[:], rmatT[:])
                for d_ in range(1, NDC):
                    nc.sync.dma_start(x_sb[d_][:], xT[d_ * P : (d_ + 1) * P, :])
                    nc.sync.dma_start(w_sb["q", d_][:], wqT[d_ * P : (d_ + 1) * P, :])
                for d_ in range(NDC):
                    nc.sync.dma_start(w_sb["k", d_][:], wkT[d_ * P : (d_ + 1) * P, :])
                for h in range(GH):
                    zo = (1 - h % 2) * HD  # zero the other head's half
                    for qt in range(NQT):
                        nc.sync.dma_start(KT[h][qt][zo : zo + HD, :], zpad[:])
                for d_ in range(NDC):
                    nc.sync.dma_start(w_sb["v", d_][:], wvT[d_ * P : (d_ + 1) * P, :])
                for kc in range(NKC):
                    nc.sync.dma_start(
                        VA[kc][:].rearrange("p (h c) -> p h c", c=HD + 1)[
                            :, :, HD : HD + 1
                        ],
                        vones[:],
                    )
                nc.sync.dma_start(dm_sb[:], dmask[:])
                nc.sync.dma_start(ones1[:], onesd[:])
                for c in range(2):
                    nc.sync.dma_start(wo_sb[c][:], woT[c * P : (c + 1) * P, :])

                # ---- Q/K projections: 4-bank groups so RoPE (DVE) drains
                # one group while the PE fills the next ----
                def rope_group(wname, dst, m, ps):
                    for st in range(NQT):
                        p_ = ps[st]
                        sl = slice(st * QW, (st + 1) * QW)
                        raw = qkvp.tile(
                            [P, QW], mdt, tag="raw", bufs=3,
                            name=f"raw{wname}{m}{st}",
                        )
                        nc.vector.tensor_copy(raw[:], p_[:])
                        # rotate_half on the PE: p_ <- R @ raw (in place)
                        nc.tensor.matmul(
                            p_[:], lhsT=rm_sb[:], rhs=raw[:], start=True, stop=True
                        )
                        m1 = qkvp.tile(
                            [P, QW], f32, tag="m1", bufs=3,
                            name=f"m1{wname}{m}{st}",
                        )
                        nc.vector.tensor_mul(m1[:], p_[:], sin_sb[:, sl])
                        tmp = qkvp.tile(
                            [P, QW], f32, tag="ctmp", bufs=3,
                            name=f"tmp{wname}{m}{st}",
                        )
                        nc.vector.tensor_mul(tmp[:], raw[:], cos_sb[:, sl])
                        if wname == "q":
                            nc.vector.tensor_add(dst[m][st][:], tmp[:], m1[:])
                        else:  # per-head padded K tiles
                            for j in range(2):
                                ro_ = j * HD
                                nc.vector.tensor_add(
                                    KT[2 * m + j][st][ro_ : ro_ + HD, :],
                                    tmp[ro_ : ro_ + HD, :],
                                    m1[ro_ : ro_ + HD, :],
                                )

                # Q: d-outer across all 8 banks -> PE paced by the x/wq DMA
                # stream with no per-group stalls
                psq = {}
                for m in range(2):
                    for st in range(NQT):
                        psq[m, st] = ppsum.tile(
                            [P, QW], f32, tag=f"pj{m}{st}", name=f"psq{m}{st}"
                        )
                for d_ in range(NDC):
                    for m in range(2):
                        for st in range(NQT):
                            nc.tensor.matmul(
                                psq[m, st][:],
                                lhsT=w_sb["q", d_][:, m * P : (m + 1) * P],
                                rhs=x_sb[d_][:, st * QW : (st + 1) * QW],
                                start=(d_ == 0),
                                stop=(d_ == NDC - 1),
                            )
                for m in range(2):
                    rope_group("q", QT, m, [psq[m, st] for st in range(NQT)])

                # K: two 4-bank groups so the first group's RoPE overlaps the
                # second group's matmuls
                for m in range(2):
                    ps = [
                        ppsum.tile(
                            [P, QW], f32, tag=f"pj{m}{st}", name=f"psk{m}{st}"
                        )
                        for st in range(NQT)
                    ]
                    for d_ in range(NDC):
                        for st in range(NQT):
                            nc.tensor.matmul(
                                ps[st][:],
                                lhsT=w_sb["k", d_][:, m * P : (m + 1) * P],
                                rhs=x_sb[d_][:, st * QW : (st + 1) * QW],
                                start=(d_ == 0),
                                stop=(d_ == NDC - 1),
                            )
                    rope_group("k", KT, m, ps)

                # ---- V projection (4-bank groups of 4 seq tiles) ----
                for g0 in (0, 4, 8, 12):
                    grp = (g0 // 4) % 2
                    psv = {
                        st: ppsum.tile(
                            [P, GD], f32, tag=f"pj{grp}{st - g0}", name=f"psv{st}"
                        )
                        for st in range(g0, g0 + 4)
                    }
                    for d_ in range(NDC):
                        for st in range(g0, g0 + 4):
                            nc.tensor.matmul(
                                psv[st][:],
                                lhsT=x_sb[d_][:, st * P : (st + 1) * P],
                                rhs=w_sb["v", d_][:],
                                start=(d_ == 0),
                                stop=(d_ == NDC - 1),
                            )
                    for st in range(g0, g0 + 4):
                        va = VA[st]
                        dst_ap = va[:].rearrange("p (h c) -> p h c", c=HD + 1)[
                            :, :, 0:HD
                        ]
                        src_ap = psv[st][:].rearrange("p (h c) -> p h c", c=HD)
                        nc.scalar.activation(dst_ap, src_ap, AF_Copy)

            # ---- attention (+ output projection sharing the PSUM pool) ----
            SEG = QW  # 512-wide exp segments, 4 rotating PSUM slots
            with (
                tc.tile_pool(name="work", bufs=1) as workp,
                tc.tile_pool(name="apsum", bufs=1, space="PSUM") as apsum,
            ):
                attnT = [
                    [
                        workp.tile(
                            [P, QW], mdt, tag=f"at{th}_{qt}", name=f"at{th}_{qt}"
                        )
                        for qt in range(NQT)
                    ]
                    for th in range(2)
                ]
                def divide_store(h, th, ro, qt, pv):
                    # normalize: ln of the denom row (ACT, reads PSUM),
                    # broadcast via ones-outer-product, 1/d = exp(-ln d)
                    dn = workp.tile([1, QW], mdt, tag="dn", bufs=2, name=f"dn{h}{qt}")
                    nc.vector.tensor_copy(dn[:], pv[qt][HD : HD + 1, :])
                    bc_ps = apsum.tile(
                        [HD, QW], f32, tag="sc", bufs=2, name=f"bcp{h}{qt}"
                    )
                    nc.tensor.matmul(
                        bc_ps[:], lhsT=ones1[:], rhs=dn[:], start=True, stop=True
                    )
                    bc1 = workp.tile(
                        [HD, QW], f32, tag="bc1", bufs=2, name=f"bc1{h}{qt}"
                    )
                    nc.scalar.activation(bc1[:], bc_ps[:], Ln)
                    bc = workp.tile([HD, QW], f32, tag="bc", bufs=2, name=f"bc{h}{qt}")
                    nc.scalar.activation(bc[:], bc1[:], Exp, scale=-1.0)
                    nc.vector.tensor_mul(
                        attnT[th][qt][ro : ro + HD, :], pv[qt][0:HD, :], bc[:]
                    )

                def emit_pv(plist, h, th, ro, pv):
                    for qt, ppt, qsa, q0, vb, ka, kb in plist:
                        nc.tensor.matmul(
                            pv[qt][0 : HD + 1, qsa - q0 :],
                            lhsT=VA[ka][:, h * (HD + 1) : (h + 1) * (HD + 1)],
                            rhs=ppt[:, qsa - q0 : QW],
                            start=(ka == 0),
                            stop=(ka == 4 * qt + 3),
                        )
                        if vb:
                            qsb = max(q0, kb * P)
                            nc.tensor.matmul(
                                pv[qt][0 : HD + 1, qsb - q0 :],
                                lhsT=VA[kb][
                                    :, h * (HD + 1) : (h + 1) * (HD + 1)
                                ],
                                rhs=ppt[:, QW + qsb - q0 :],
                                start=False,
                                stop=(kb == 4 * qt + 3),
                            )
                            if kb == 4 * qt + 3:
                                divide_store(h, th, ro, qt, pv)

                for h in range(GH):
                    th, ro = h // 2, (h % 2) * HD
                    pv = [
                        apsum.tile([P, QW], f32, tag=f"pv{qt}", name=f"pv{h}_{qt}")
                        for qt in range(NQT)
                    ]
                    # kc-PAIR segments: one [128,1024] scores tile = two
                    # k-chunks of one q-tile -> one exp per pair. 1-pair skew
                    # keeps the in-order PE stream clear of the exp chain.
                    pending = None
                    for pk in range(NKC // 2):
                        ka, kb = 2 * pk, 2 * pk + 1
                        k0a, k0b = ka * P, kb * P
                        cur = []
                        for qt in range(NQT):
                            q0 = qt * QW
                            if k0a >= q0 + QW:
                                continue
                            qsa = max(q0, k0a)
                            vb = k0b < q0 + QW
                            sps = apsum.tile(
                                [P, 2 * QW], f32, tag="sc", bufs=2,
                                name=f"sc{h}_{pk}_{qt}",
                            )
                            nc.tensor.matmul(
                                sps[:, qsa - q0 : QW],
                                lhsT=KT[h][ka // 4][
                                    :, (k0a % QW) : (k0a % QW) + P
                                ],
                                rhs=QT[th][qt][:, qsa - q0 :],
                                start=True,
                                stop=True,
                            )
                            if vb:
                                qsb = max(q0, k0b)
                                nc.tensor.matmul(
                                    sps[:, QW + qsb - q0 :],
                                    lhsT=KT[h][kb // 4][
                                        :, (k0b % QW) : (k0b % QW) + P
                                    ],
                                    rhs=QT[th][qt][:, qsb - q0 :],
                                    start=True,
                                    stop=True,
                                )
                            pt = workp.tile(
                                [P, 2 * QW], mdt, tag="pt", bufs=4,
                                name=f"pt{h}_{pk}_{qt}",
                            )
                            off = qsa - q0
                            if not vb:
                                nc.scalar.activation(
                                    pt[:, off:QW], sps[:, off:QW], Exp, scale=SCALE
                                )
                            elif k0b <= q0:  # contiguous, no hole
                                nc.scalar.activation(
                                    pt[:, off:], sps[:, off:], Exp, scale=SCALE
                                )
                            else:  # hole between the halves: two ranges
                                nc.scalar.activation(
                                    pt[:, off:QW], sps[:, off:QW], Exp, scale=SCALE
                                )
                                ob2 = QW + k0b - q0
                                nc.scalar.activation(
                                    pt[:, ob2:], sps[:, ob2:], Exp, scale=SCALE
                                )
                            if k0a >= q0:  # ka diagonal block
                                nc.vector.tensor_mul(
                                    pt[:, off : off + P],
                                    pt[:, off : off + P],
                                    dm_sb[:],
                                )
                            if vb and k0b >= q0:  # kb diagonal block
                                ob_ = QW + k0b - q0
                                nc.vector.tensor_mul(
                                    pt[:, ob_ : ob_ + P],
                                    pt[:, ob_ : ob_ + P],
                                    dm_sb[:],
                                )
                            cur.append((qt, pt, qsa, q0, vb, ka, kb))
                        if pending is not None:
                            emit_pv(pending, h, th, ro, pv)
                        pending = cur
                    emit_pv(pending, h, th, ro, pv)

                # ---- output projection (PSUM slots reuse the pv tags) ----
                for st in range(NST):
                    for n in range(2):
                        ops = apsum.tile(
                            [P, QW], f32, tag=f"pv{(2 * st + n) % 4}",
                            name=f"op{st}_{n}",
                        )
                        for c in range(2):
                            nc.tensor.matmul(
                                ops[:],
                                lhsT=attnT[c][st // 4][
                                    :, (st % 4) * P : (st % 4 + 1) * P
                                ],
                                rhs=wo_sb[c][:, n * QW : (n + 1) * QW],
                                start=(c == 0),
                                stop=(c == 1),
                            )
                        ob = workp.tile(
                            [P, QW], f32, tag="ob", bufs=4, name=f"ob{st}_{n}"
                        )
                        if n == 0:
                            nc.vector.tensor_copy(ob[:], ops[:])
                        else:
                            nc.scalar.activation(ob[:], ops[:], AF_Copy)
                        nc.sync.dma_start(
                            out[st * P : (st + 1) * P, n * QW : (n + 1) * QW],
                            ob[:],
                        )

    if split_waits:
        nsplit = _split_excess_waits(nc)
        if nsplit:
            print(f"[kernel] split {nsplit} excess-wait NOPs")
    _cache[key] = nc
    return nc


def _rope_tables():
    inv = 1.0 / (10000.0 ** (np.arange(0, HD, 2, dtype=np.float32) / HD))  # [32]
    t = np.arange(S, dtype=np.float32)
    freqs = np.outer(inv, t)  # [32, S]
    cosb = np.cos(freqs).astype(np.float32)
    sinb = np.sin(freqs).astype(np.float32)
    cosT = np.concatenate([cosb, cosb], axis=0)  # [64, S]
    sinT = np.concatenate([sinb, sinb], axis=0)
    return np.tile(cosT, (2, 1)), np.tile(sinT, (2, 1))  # [128, S]


def _rot_matrix():
    # R @ q  ==  rotate_half(q) per 64-row head block (sign included)
    R = np.zeros((P, P), dtype=np.float32)
    for b in range(2):
        for j in range(32):
            R[b * 64 + j, b * 64 + j + 32] = -1.0
            R[b * 64 + j + 32, b * 64 + j] = 1.0
    return np.ascontiguousarray(R.T)


def make_in_maps(x, Wq, Wk, Wv, Wo):
    x = np.ascontiguousarray(np.asarray(x, dtype=np.float32))
    Wq, Wk, Wv, Wo = (np.asarray(w, dtype=np.float32) for w in (Wq, Wk, Wv, Wo))
    cos2, sine = _rope_tables()
    kk = np.arange(P)[:, None]
    qq = np.arange(P)[None, :]
    dmask = (kk <= qq).astype(np.float32)
    in_maps = []
    for c in range(NCORES):
        b, g = c // GH, c % GH
        sl = slice(g * GD, (g + 1) * GD)
        in_maps.append(
            {
                "xT": np.ascontiguousarray(x[b].T),
                "wqT": np.ascontiguousarray(Wq[sl, :].T),
                "wkT": np.ascontiguousarray(Wk[sl, :].T),
                "wvT": np.ascontiguousarray(Wv[sl, :].T),
                "woT": np.ascontiguousarray(Wo[:, sl].T),
                "cos2": cos2,
                "sine": sine,
                "dmask": dmask,
                "onesd": np.ones((1, HD), dtype=np.float32),
                "vones": np.ones((P, GH, 1), dtype=np.float32),
                "zpad": np.zeros((HD, QW), dtype=np.float32),
                "rmatT": _rot_matrix(),
            }
        )
    return in_maps


def run(x, Wq, Wk, Wv, Wo, trace=False):
    from concourse.bass_utils import run_bass_kernel_spmd

    nc = build_nc()
    in_maps = make_in_maps(x, Wq, Wk, Wv, Wo)
    res = run_bass_kernel_spmd(nc, in_maps, list(range(NCORES)), trace=trace)
    partials = [res.results[c]["out"] for c in range(NCORES)]
    full = np.zeros((B, S, D), dtype=np.float32)
    for c in range(NCORES):
        full[c // GH] += partials[c]
    return full, res


def kernel(x, Wq, Wk, Wv, Wo):
    full, _ = run(x, Wq, Wk, Wv, Wo, trace=False)
    return full
